# revision 9
# baseline (speedup 1.0000x reference)
"""ChannelMerger kernel for Trainium2, data-parallel over batch on 8 NeuronCores.

Reference computation (identical for every batch b, since layout is
batch-independent):
    pos       = layout + 0.2                              # [C, 2]
    loc[c,ij] = (2*pi/1.4) * (i * pos_x[c] + j * pos_y[c])   (i = ij>>5, j = ij&31)
    emb       = [cos(loc), sin(loc)]                      # [C, D=2048]
    weights   = softmax(emb @ heads.T, axis=C)            # [O, C]
    out[b]    = weights @ x[b]                            # [O, T]

Device program (identical on all 8 cores, each owns 8 batches):
  phase 1 (replicated, bf16): embT[d, c] built via one-op range reduction
    h = (f + 0.5) mod 1 and ACT Sin(2*pi*h - pi); cos via the +0.25 phase
    shift.  headsT arrives pre-transposed/bf16 from the host, so scoresT =
    embT.T @ headsT needs no on-device transpose.  softmax normalization is
    folded into the weights (w = exp * recip broadcast along partitions), so
    phase 2 has no post-matmul scaling.
  phase 2: x is staged [C, B_LOC*T] bf16 (b-major t columns); out.T tiles
    [t=128, O] accumulate over 3 c-chunks per 128-column t-chunk; 125 chunks,
    grouped 5 per out tile so out DMAs carry contiguous 2.7 KiB
    per-partition descriptors.  Output layout [25, 128, 5*270] bf16 is
    unscrambled on the host.
"""

import sys

for _p in ("/opt/trn_rl_repo", "/root/.axon_site/_ro/trn_rl_repo"):
    if _p not in sys.path:
        sys.path.append(_p)

import numpy as np

B, C, T = 64, 270, 2000
O, D = 270, 2048
N_CORES = 8
B_LOC = B // N_CORES          # 8 batches per core
TN = B_LOC * T                # 16000 t-columns per core, b-major
NTC = TN // 128               # 125 t-chunks
TCA = 64                      # t-chunks resident in the first x half-tile
GRP = 5                       # t-chunks per out tile / out DMA
NG = NTC // GRP               # 25 out DMAs
NF = 32
MARGIN = 0.2
WIDTH = 1.0 + 2.0 * MARGIN    # 1.4

C_CHUNKS = [(0, 128), (128, 128), (256, 14)]
K16 = 16                      # d-chunks of 128 (D = 2048)
KK8 = 8                       # ij-chunks of 128 (D/2 = 1024)

_cache = {}


def _build():
    import concourse.tile as tile
    from concourse import bacc, mybir

    F32 = mybir.dt.float32
    F32R = mybir.dt.float32r
    BF16 = mybir.dt.bfloat16
    ACT = mybir.ActivationFunctionType
    ALU = mybir.AluOpType
    TWO_PI = float(2.0 * np.pi)
    PI = float(np.pi)

    nc = bacc.Bacc("TRN2", target_bir_lowering=False, debug=False,
                   num_devices=N_CORES)

    x_ap = nc.dram_tensor("x", [C, TN], BF16, kind="ExternalInput").ap()
    lay_ap = nc.dram_tensor("layout", [C, 2], F32, kind="ExternalInput").ap()
    # headsT[p, k, o] = heads[o, k*128 + p], bf16 (host pre-permuted)
    ht_ap = nc.dram_tensor("headsT", [128, K16 * O], BF16,
                           kind="ExternalInput").ap()
    # host-precomputed per-partition index constants:
    #   cols 0..7  i_k = (k*128+p)>>5;  col 8  j = p&31
    ijc_ap = nc.dram_tensor("ijc", [128, 9], F32, kind="ExternalInput").ap()
    # out[g, p, s*270 + o] = out_T[(g*GRP + s)*128 + p, o]
    out_ap = nc.dram_tensor("out", [NG, 128, GRP * O], BF16,
                            kind="ExternalOutput").ap()

    with tile.TileContext(nc) as tc:
        with tc.tile_pool(name="xin", bufs=1) as xpool, \
             tc.tile_pool(name="oout", bufs=10) as opool, \
             tc.tile_pool(name="wts", bufs=1) as wpool, \
             tc.tile_pool(name="const", bufs=1) as cpool:

            # ---- input DMAs, all on the sync (SP) queue so the shared DMA
            # engines serve them in priority order: the tiny phase-1 inputs
            # first (they head the dependency chain), then headsT (needed by
            # the scores matmuls), then the x halves.
            posx = cpool.tile([1, C], F32)
            posy = cpool.tile([1, C], F32)
            nc.sync.dma_start(posx[:], lay_ap[:, 0])
            nc.sync.dma_start(posy[:], lay_ap[:, 1])
            ijc = cpool.tile([128, 9], F32)
            nc.sync.dma_start(ijc[:], ijc_ap[:])

            headsT = cpool.tile([128, K16 * O], BF16)
            nc.sync.dma_start(headsT[:], ht_ap[:])

            xa = []
            xb = []
            for cc, (c0, csz) in enumerate(C_CHUNKS):
                t = xpool.tile([128, TCA * 128], BF16, tag=f"xa{cc}",
                               name=f"xa{cc}")
                nc.sync.dma_start(t[:csz, :], x_ap[c0:c0 + csz, :TCA * 128])
                xa.append(t)
            for cc, (c0, csz) in enumerate(C_CHUNKS):
                t = xpool.tile([128, TN - TCA * 128], BF16, tag=f"xb{cc}",
                               name=f"xb{cc}")
                nc.sync.dma_start(t[:csz, :], x_ap[c0:c0 + csz, TCA * 128:])
                xb.append(t)

            # ---- u, v in fractional turns (one-time setup; keeps the
            # per-chunk loop free of bias adds)
            u_row = cpool.tile([1, C], F32)
            nc.vector.tensor_scalar(u_row[:], posx[:], MARGIN, 1.0 / WIDTH,
                                    ALU.add, ALU.mult)
            v_row = cpool.tile([1, C], F32)
            nc.vector.tensor_scalar(v_row[:], posy[:], MARGIN, 1.0 / WIDTH,
                                    ALU.add, ALU.mult)
            u_bc = cpool.tile([128, C], F32)
            nc.gpsimd.partition_broadcast(u_bc[:], u_row[:])
            v_bc = cpool.tile([128, C], F32)
            nc.gpsimd.partition_broadcast(v_bc[:], v_row[:])
            t2 = cpool.tile([128, C], F32)
            nc.gpsimd.tensor_scalar(t2[:], v_bc[:], ijc[:, 8:9], None,
                                    ALU.mult)

            # persistent weights for phase 2
            w = [wpool.tile([128, O], BF16, tag=f"w{i}", name=f"w{i}")
                 for i in range(3)]
            ones = wpool.tile([128, 1], BF16)
            nc.vector.memset(ones[:], 1.0)
            recip_bch = wpool.tile([128, O], BF16)

            with tc.tile_pool(name="emb", bufs=1) as epool, \
                 tc.tile_pool(name="fwork", bufs=3) as fpool, \
                 tc.tile_pool(name="hwork", bufs=4) as hpool, \
                 tc.tile_pool(name="sc_psum", bufs=1, space="PSUM") as scp, \
                 tc.tile_pool(name="sum_psum", bufs=1, space="PSUM") as sup, \
                 tc.tile_pool(name="expf", bufs=1) as xfp:

                # embT[k][p, c]: k < 8 cos chunks, k >= 8 sin chunks.
                # h = i*u + j*v; q = round(h) via the magic-constant trick;
                # the Sin argument 2*pi*(h - q) stays inside the accurate
                # [-pi, pi] domain.  cos chunks shift h by +0.25 turns.
                MAGIC = 12582912.0  # 1.5 * 2**23: ulp 1 -> add/sub rounds to int
                embT = [epool.tile([128, C], BF16, tag=f"embT{i}",
                                   name=f"embT{i}") for i in range(K16)]
                for kk in range(KK8):
                    hs = fpool.tile([128, C], F32, tag="hs", bufs=2,
                                    name="hs")
                    nc.vector.scalar_tensor_tensor(
                        hs[:], u_bc[:], ijc[:, kk:kk + 1], t2[:],
                        ALU.mult, ALU.add)
                    hc = fpool.tile([128, C], F32, tag="hc", bufs=2,
                                    name="hc")
                    nc.vector.tensor_scalar(hc[:], hs[:], 0.25, None,
                                            ALU.add)
                    for ci, h in ((KK8, hs), (0, hc)):
                        q = hpool.tile([128, C], F32, tag="q", bufs=4,
                                       name="q")
                        nc.gpsimd.tensor_scalar(q[:], h[:], MAGIC, MAGIC,
                                                ALU.add, ALU.subtract)
                        fs = fpool.tile([128, C], F32, tag="fs", bufs=4,
                                        name="fs")
                        nc.vector.tensor_tensor(fs[:], h[:], q[:],
                                                ALU.subtract)
                        nc.scalar.activation(embT[ci + kk][:], fs[:], ACT.Sin,
                                             scale=TWO_PI)

                # scoresT[c, o] accumulated over the 16 d-chunks, in the
                # order the embT tiles are produced (cos_k, sin_k pairs)
                sc = [scp.tile([128, O], F32, tag=f"sc{i}", name=f"sc{i}")
                      for i in range(3)]
                k_seq = [k for kk in range(KK8) for k in (kk, KK8 + kk)]
                for ki, k in enumerate(k_seq):
                    for cc, (c0, csz) in enumerate(C_CHUNKS):
                        nc.tensor.matmul(sc[cc][:csz, :],
                                         embT[k][:, c0:c0 + csz],
                                         headsT[:, k * O:(k + 1) * O],
                                         start=(ki == 0),
                                         stop=(ki == K16 - 1))

                # w = exp(scores) straight to bf16, then normalized in
                # place once the ones-matmul sums and reciprocal land.
                for cc, (c0, csz) in enumerate(C_CHUNKS):
                    nc.scalar.activation(w[cc][:csz, :], sc[cc][:csz, :],
                                         ACT.Exp)
                ssum = sup.tile([1, O], F32, tag="ssum", name="ssum")
                for cc, (c0, csz) in enumerate(C_CHUNKS):
                    nc.tensor.matmul(ssum[:, :], ones[:csz, :],
                                     w[cc][:csz, :],
                                     start=(cc == 0), stop=(cc == 2))
                recip_rowh = xfp.tile([1, O], BF16)
                with nc.allow_low_precision(
                        reason="bf16 softmax recip; rel-err gate is 2e-2"):
                    nc.vector.reciprocal(recip_rowh[:], ssum[:, :])
                nc.gpsimd.partition_broadcast(recip_bch[:], recip_rowh[:])
                for cc, (c0, csz) in enumerate(C_CHUNKS):
                    nc.vector.tensor_tensor(w[cc][:csz, :], w[cc][:csz, :],
                                            recip_bch[:csz, :], ALU.mult)

            # ---- phase 2: out_T[t, o] = x[:, t-chunk].T @ w, 125 chunks.
            # PSUM->SBUF drains alternate DVE / ACT copies (gpsimd cannot
            # read PSUM); out DMAs alternate the sync HWDGE queue and the
            # Pool SWDGE queue.
            with tc.tile_pool(name="mm_psum", bufs=8, space="PSUM") as mmp:
                for g in range(NG):
                    ot = opool.tile([128, GRP * O], BF16, tag="ot", name="ot")
                    for s in range(GRP):
                        tci = g * GRP + s
                        ps = mmp.tile([128, O], F32, tag="mm", name="mm")
                        if tci < TCA:
                            src, col0 = xa, tci * 128
                        else:
                            src, col0 = xb, (tci - TCA) * 128
                        for cc, (c0, csz) in enumerate(C_CHUNKS):
                            nc.tensor.matmul(ps[:, :],
                                             src[cc][:csz, col0:col0 + 128],
                                             w[cc][:csz, :],
                                             start=(cc == 0), stop=(cc == 2))
                        dst = ot[:, s * O:(s + 1) * O]
                        if tci % 2 == 0:
                            nc.vector.tensor_copy(dst, ps[:, :])
                        else:
                            nc.scalar.activation(dst, ps[:, :], ACT.Copy)
                    if g == NG - 1:
                        # fine-grained final stores on three queues shorten
                        # the drain tail
                        q_eng = [nc.scalar, nc.sync, nc.gpsimd]
                        for s in range(GRP):
                            q_eng[s % 3].dma_start(
                                out_ap[g][:, s * O:(s + 1) * O],
                                ot[:, s * O:(s + 1) * O])
                    elif g % 2 == 0:
                        nc.sync.dma_start(out_ap[g], ot[:])
                    else:
                        nc.gpsimd.dma_start(out_ap[g], ot[:])

    nc.compile()
    return nc


def _ijc_const():
    p = np.arange(128)
    cols = [((k * 128 + p) >> 5).astype(np.float64) for k in range(KK8)]
    cols.append((p & 31).astype(np.float64))
    return np.stack(cols, axis=1).astype(np.float32)


def get_nc():
    if "nc" not in _cache:
        _cache["nc"] = _build()
    return _cache["nc"]


def _prep_inputs(x, layout, heads):
    """Host-side staging: bf16 casts + device-friendly layouts."""
    import ml_dtypes
    BF16 = ml_dtypes.bfloat16
    ijc = _ijc_const()
    headsT = np.ascontiguousarray(
        heads.astype(np.float32).T.reshape(K16, 128, O).transpose(1, 0, 2)
        .reshape(128, K16 * O)).astype(BF16)
    lay = np.ascontiguousarray(layout.astype(np.float32))
    in_maps = []
    for m in range(N_CORES):
        xs = np.ascontiguousarray(
            x[m * B_LOC:(m + 1) * B_LOC].astype(np.float32)
            .transpose(1, 0, 2).reshape(C, TN)).astype(BF16)
        in_maps.append({"x": xs, "layout": lay, "headsT": headsT, "ijc": ijc})
    return in_maps


def _unscramble_out(r):
    """[NG, 128, GRP*O] bf16 -> [B_LOC, O, T] f32."""
    r = np.asarray(r).reshape(NG, 128, GRP, O).transpose(0, 2, 1, 3)
    return np.ascontiguousarray(
        r.reshape(B_LOC, T, O).transpose(0, 2, 1)).astype(np.float32)


def kernel(x, layout, heads):
    from concourse.bass_utils import run_bass_kernel_spmd

    assert x.shape == (B, C, T) and layout.shape == (C, 2)
    assert heads.shape == (O, D)
    nc = get_nc()
    in_maps = _prep_inputs(x, layout, heads)
    res = run_bass_kernel_spmd(nc, in_maps, list(range(N_CORES)))
    out = np.concatenate(
        [_unscramble_out(res.results[m]["out"]) for m in range(N_CORES)],
        axis=0)
    return out


# revision 11
# speedup vs baseline: 1.0273x; 1.0273x over previous
"""ChannelMerger kernel for Trainium2, data-parallel over batch on 8 NeuronCores.

Reference computation (identical for every batch b, since layout is
batch-independent):
    pos       = layout + 0.2                              # [C, 2]
    loc[c,ij] = (2*pi/1.4) * (i * pos_x[c] + j * pos_y[c])   (i = ij>>5, j = ij&31)
    emb       = [cos(loc), sin(loc)]                      # [C, D=2048]
    weights   = softmax(emb @ heads.T, axis=C)            # [O, C]
    out[b]    = weights @ x[b]                            # [O, T]

Device program (identical on all 8 cores, each owns 8 batches):
  phase 1 (replicated, bf16): embT[d, c] built via one-op range reduction
    h = (f + 0.5) mod 1 and ACT Sin(2*pi*h - pi); cos via the +0.25 phase
    shift.  headsT arrives pre-transposed/bf16 from the host, so scoresT =
    embT.T @ headsT needs no on-device transpose.  softmax normalization is
    folded into the weights (w = exp * recip broadcast along partitions), so
    phase 2 has no post-matmul scaling.
  phase 2: x is staged [C, B_LOC*T] bf16 (b-major t columns); out.T tiles
    [t=128, O] accumulate over 3 c-chunks per 128-column t-chunk; 125 chunks,
    grouped 5 per out tile so out DMAs carry contiguous 2.7 KiB
    per-partition descriptors.  Output layout [25, 128, 5*270] bf16 is
    unscrambled on the host.
"""

import sys

for _p in ("/opt/trn_rl_repo", "/root/.axon_site/_ro/trn_rl_repo"):
    if _p not in sys.path:
        sys.path.append(_p)

import numpy as np

B, C, T = 64, 270, 2000
O, D = 270, 2048
N_CORES = 8
B_LOC = B // N_CORES          # 8 batches per core
TN = B_LOC * T                # 16000 t-columns per core, b-major
NTC = TN // 128               # 125 t-chunks
TCA = 64                      # t-chunks resident in the first x half-tile
GRP = 5                       # t-chunks per out tile / out DMA
NG = NTC // GRP               # 25 out DMAs
NF = 32
MARGIN = 0.2
WIDTH = 1.0 + 2.0 * MARGIN    # 1.4

C_CHUNKS = [(0, 128), (128, 128), (256, 14)]
K16 = 16                      # d-chunks of 128 (D = 2048)
KK8 = 8                       # ij-chunks of 128 (D/2 = 1024)

_cache = {}


def _build():
    import concourse.tile as tile
    from concourse import bacc, mybir

    F32 = mybir.dt.float32
    F32R = mybir.dt.float32r
    BF16 = mybir.dt.bfloat16
    ACT = mybir.ActivationFunctionType
    ALU = mybir.AluOpType
    TWO_PI = float(2.0 * np.pi)
    PI = float(np.pi)

    nc = bacc.Bacc("TRN2", target_bir_lowering=False, debug=False,
                   num_devices=N_CORES)

    x_ap = nc.dram_tensor("x", [C, TN], BF16, kind="ExternalInput").ap()
    lay_ap = nc.dram_tensor("layout", [C, 2], F32, kind="ExternalInput").ap()
    # headsT[p, k, o] = heads[o, k*128 + p], bf16 (host pre-permuted)
    ht_ap = nc.dram_tensor("headsT", [128, K16 * O], BF16,
                           kind="ExternalInput").ap()
    # host-precomputed per-partition index constants:
    #   cols 0..7  i_k = (k*128+p)>>5;  col 8  j = p&31
    ijc_ap = nc.dram_tensor("ijc", [128, 9], F32, kind="ExternalInput").ap()
    # out[g, p, s*270 + o] = out_T[(g*GRP + s)*128 + p, o]
    out_ap = nc.dram_tensor("out", [NG, 128, GRP * O], BF16,
                            kind="ExternalOutput").ap()

    with tile.TileContext(nc) as tc:
        with tc.tile_pool(name="xin", bufs=1) as xpool, \
             tc.tile_pool(name="oout", bufs=10) as opool, \
             tc.tile_pool(name="wts", bufs=1) as wpool, \
             tc.tile_pool(name="const", bufs=1) as cpool:

            # ---- input DMAs, all on the sync (SP) queue so the shared DMA
            # engines serve them in priority order: the tiny phase-1 inputs
            # first (they head the dependency chain), then headsT (needed by
            # the scores matmuls), then the x halves.
            posx = cpool.tile([1, C], F32)
            posy = cpool.tile([1, C], F32)
            nc.sync.dma_start(posx[:], lay_ap[:, 0])
            nc.sync.dma_start(posy[:], lay_ap[:, 1])
            ijc = cpool.tile([128, 9], F32)
            nc.sync.dma_start(ijc[:], ijc_ap[:])

            headsT = cpool.tile([128, K16 * O], BF16)
            nc.sync.dma_start(headsT[:], ht_ap[:])

            xa = []
            xb = []
            for cc, (c0, csz) in enumerate(C_CHUNKS):
                t = xpool.tile([128, TCA * 128], BF16, tag=f"xa{cc}",
                               name=f"xa{cc}")
                nc.sync.dma_start(t[:csz, :], x_ap[c0:c0 + csz, :TCA * 128])
                xa.append(t)
            for cc, (c0, csz) in enumerate(C_CHUNKS):
                t = xpool.tile([128, TN - TCA * 128], BF16, tag=f"xb{cc}",
                               name=f"xb{cc}")
                nc.sync.dma_start(t[:csz, :], x_ap[c0:c0 + csz, TCA * 128:])
                xb.append(t)

            # ---- u, v in fractional turns (one-time setup; keeps the
            # per-chunk loop free of bias adds)
            u_row = cpool.tile([1, C], F32)
            nc.vector.tensor_scalar(u_row[:], posx[:], MARGIN, 1.0 / WIDTH,
                                    ALU.add, ALU.mult)
            v_row = cpool.tile([1, C], F32)
            nc.vector.tensor_scalar(v_row[:], posy[:], MARGIN, 1.0 / WIDTH,
                                    ALU.add, ALU.mult)
            u_bc = cpool.tile([128, C], F32)
            nc.gpsimd.partition_broadcast(u_bc[:], u_row[:])
            v_bc = cpool.tile([128, C], F32)
            nc.gpsimd.partition_broadcast(v_bc[:], v_row[:])
            t2 = cpool.tile([128, C], F32)
            nc.gpsimd.tensor_scalar(t2[:], v_bc[:], ijc[:, 8:9], None,
                                    ALU.mult)

            # persistent weights for phase 2
            w = [wpool.tile([128, O], BF16, tag=f"w{i}", name=f"w{i}")
                 for i in range(3)]
            ones = wpool.tile([128, 1], BF16)
            nc.vector.memset(ones[:], 1.0)
            recip_bch = wpool.tile([128, O], BF16)

            with tc.tile_pool(name="emb", bufs=1) as epool, \
                 tc.tile_pool(name="fwork", bufs=3) as fpool, \
                 tc.tile_pool(name="hwork", bufs=4) as hpool, \
                 tc.tile_pool(name="sc_psum", bufs=1, space="PSUM") as scp, \
                 tc.tile_pool(name="sum_psum", bufs=1, space="PSUM") as sup, \
                 tc.tile_pool(name="expf", bufs=1) as xfp:

                # embT[k][p, c]: k < 8 cos chunks, k >= 8 sin chunks.
                # h = i*u + j*v; q = round(h) via the magic-constant trick;
                # the Sin argument 2*pi*(h - q) stays inside the accurate
                # [-pi, pi] domain.  cos chunks shift h by +0.25 turns.
                MAGIC = 12582912.0  # 1.5 * 2**23: ulp 1 -> add/sub rounds to int
                embT = [epool.tile([128, C], BF16, tag=f"embT{i}",
                                   name=f"embT{i}") for i in range(K16)]
                for kk in range(KK8):
                    hs = fpool.tile([128, C], F32, tag="hs", bufs=2,
                                    name="hs")
                    nc.vector.scalar_tensor_tensor(
                        hs[:], u_bc[:], ijc[:, kk:kk + 1], t2[:],
                        ALU.mult, ALU.add)
                    hc = fpool.tile([128, C], F32, tag="hc", bufs=2,
                                    name="hc")
                    nc.vector.tensor_scalar(hc[:], hs[:], 0.25, None,
                                            ALU.add)
                    for ci, h in ((KK8, hs), (0, hc)):
                        q = hpool.tile([128, C], F32, tag="q", bufs=4,
                                       name="q")
                        nc.gpsimd.tensor_scalar(q[:], h[:], MAGIC, MAGIC,
                                                ALU.add, ALU.subtract)
                        fs = fpool.tile([128, C], F32, tag="fs", bufs=4,
                                        name="fs")
                        nc.vector.tensor_tensor(fs[:], h[:], q[:],
                                                ALU.subtract)
                        nc.scalar.activation(embT[ci + kk][:], fs[:], ACT.Sin,
                                             scale=TWO_PI)

                # scoresT[c, o] accumulated over the 16 d-chunks, in the
                # order the embT tiles are produced (cos_k, sin_k pairs)
                sc = [scp.tile([128, O], F32, tag=f"sc{i}", name=f"sc{i}")
                      for i in range(3)]
                k_seq = [k for kk in range(KK8) for k in (kk, KK8 + kk)]
                for ki, k in enumerate(k_seq):
                    for cc, (c0, csz) in enumerate(C_CHUNKS):
                        nc.tensor.matmul(sc[cc][:csz, :],
                                         embT[k][:, c0:c0 + csz],
                                         headsT[:, k * O:(k + 1) * O],
                                         start=(ki == 0),
                                         stop=(ki == K16 - 1))

                # w = exp(scores) straight to bf16, then normalized in
                # place once the ones-matmul sums and reciprocal land.
                for cc, (c0, csz) in enumerate(C_CHUNKS):
                    nc.scalar.activation(w[cc][:csz, :], sc[cc][:csz, :],
                                         ACT.Exp)
                ssum = sup.tile([1, O], F32, tag="ssum", name="ssum")
                for cc, (c0, csz) in enumerate(C_CHUNKS):
                    nc.tensor.matmul(ssum[:, :], ones[:csz, :],
                                     w[cc][:csz, :],
                                     start=(cc == 0), stop=(cc == 2))
                recip_rowh = xfp.tile([1, O], BF16)
                with nc.allow_low_precision(
                        reason="bf16 softmax recip; rel-err gate is 2e-2"):
                    nc.vector.reciprocal(recip_rowh[:], ssum[:, :])
                nc.gpsimd.partition_broadcast(recip_bch[:], recip_rowh[:])
                for cc, (c0, csz) in enumerate(C_CHUNKS):
                    nc.vector.tensor_tensor(w[cc][:csz, :], w[cc][:csz, :],
                                            recip_bch[:csz, :], ALU.mult)

            # ---- phase 2: out_T[t, o] = x[:, t-chunk].T @ w, 125 chunks.
            # PSUM->SBUF drains alternate DVE / ACT copies (gpsimd cannot
            # read PSUM); out DMAs alternate the sync HWDGE queue and the
            # Pool SWDGE queue.
            with tc.tile_pool(name="mm_psum", bufs=8, space="PSUM") as mmp:
                for g in range(NG):
                    ot = opool.tile([128, GRP * O], BF16, tag="ot", name="ot")
                    for s in range(GRP):
                        tci = g * GRP + s
                        ps = mmp.tile([128, O], F32, tag="mm", name="mm")
                        if tci < TCA:
                            src, col0 = xa, tci * 128
                        else:
                            src, col0 = xb, (tci - TCA) * 128
                        for cc, (c0, csz) in enumerate(C_CHUNKS):
                            nc.tensor.matmul(ps[:, :],
                                             src[cc][:csz, col0:col0 + 128],
                                             w[cc][:csz, :],
                                             start=(cc == 0), stop=(cc == 2))
                        dst = ot[:, s * O:(s + 1) * O]
                        if tci % 2 == 0:
                            nc.vector.tensor_copy(dst, ps[:, :])
                        else:
                            nc.scalar.activation(dst, ps[:, :], ACT.Copy)
                    if g == NG - 1:
                        # fine-grained final stores on three queues shorten
                        # the drain tail
                        q_eng = [nc.scalar, nc.sync, nc.gpsimd]
                        for s in range(GRP):
                            q_eng[s % 3].dma_start(
                                out_ap[g][:, s * O:(s + 1) * O],
                                ot[:, s * O:(s + 1) * O])
                    elif g % 2 == 0:
                        nc.sync.dma_start(out_ap[g], ot[:])
                    else:
                        nc.gpsimd.dma_start(out_ap[g], ot[:])

    nc.compile()
    return nc


def _ijc_const():
    p = np.arange(128)
    cols = [((k * 128 + p) >> 5).astype(np.float64) for k in range(KK8)]
    cols.append((p & 31).astype(np.float64))
    return np.stack(cols, axis=1).astype(np.float32)


def get_nc():
    if "nc" not in _cache:
        _cache["nc"] = _build()
    return _cache["nc"]


def _prep_inputs(x, layout, heads):
    """Host-side staging: bf16 casts + device-friendly layouts."""
    import ml_dtypes
    BF16 = ml_dtypes.bfloat16
    ijc = _ijc_const()
    headsT = np.ascontiguousarray(
        heads.astype(np.float32).T.reshape(K16, 128, O).transpose(1, 0, 2)
        .reshape(128, K16 * O)).astype(BF16)
    lay = np.ascontiguousarray(layout.astype(np.float32))
    in_maps = []
    for m in range(N_CORES):
        xs = np.ascontiguousarray(
            x[m * B_LOC:(m + 1) * B_LOC].astype(np.float32)
            .transpose(1, 0, 2).reshape(C, TN)).astype(BF16)
        in_maps.append({"x": xs, "layout": lay, "headsT": headsT, "ijc": ijc})
    return in_maps


def _unscramble_out(r):
    """[NG, 128, GRP*O] bf16 -> [B_LOC, O, T] f32."""
    r = np.asarray(r).reshape(NG, 128, GRP, O).transpose(0, 2, 1, 3)
    return np.ascontiguousarray(
        r.reshape(B_LOC, T, O).transpose(0, 2, 1)).astype(np.float32)


def kernel(x, layout, heads):
    from concourse.bass_utils import run_bass_kernel_spmd

    assert x.shape == (B, C, T) and layout.shape == (C, 2)
    assert heads.shape == (O, D)
    nc = get_nc()
    in_maps = _prep_inputs(x, layout, heads)
    res = run_bass_kernel_spmd(nc, in_maps, list(range(N_CORES)))
    out = np.concatenate(
        [_unscramble_out(res.results[m]["out"]) for m in range(N_CORES)],
        axis=0)
    return out
